# revision 5
# baseline (speedup 1.0000x reference)
# nn_GateModLinear on 8 Trainium2 NeuronCores (Bass/Tile), data-parallel over batch.
#
# Math: z[b,i] = gW[b,i] * sum_{m,j} pW[b,m] Ws[m,i,j] x[b,j]  +  gb[b,i] * (pb@bs)[b,i]
#       out   = ELU(LayerNorm(z))
#
# Device algorithm (per core, 512 batch rows):
#   y[(m,j), b] = pW[b,m] * x[b,j]          (bf16, built on-chip from x^T and broadcast pW^T)
#   z[b, i]     = sum_k yT[k,b] * W2T[k,i]  (single K=16384 PE accumulation; W2T streamed
#                                            from DRAM via DMA-transpose of bf16 Ws tiles)
#   gating + bias + LayerNorm stats fused into the PSUM drains; ELU via Exp/min/max.
import os
import numpy as np
import ml_dtypes

B, M, DI, DO = 4096, 8, 2048, 2048
NCORES = 8
BS = B // NCORES            # 512 rows per core
P = 128
NB = BS // P                # 4 b-tiles per core
NC_I = DO // 512            # 4 output column chunks of 512
JCH = DI // P               # 16 j-chunks per expert
KCH = M * JCH               # 128 k-chunks of 128 (k = m*16 + jc)
LN_EPS = 1e-5

BF16 = ml_dtypes.bfloat16

_cache = {}


def _build_module():
    import concourse.bacc as bacc
    import concourse.mybir as mybir
    import concourse.tile as tile
    from contextlib import ExitStack

    f32 = mybir.dt.float32
    bf16 = mybir.dt.bfloat16
    FT = mybir.ActivationFunctionType
    OP = mybir.AluOpType

    nc = bacc.Bacc()
    x_t = nc.dram_tensor("x_bf", [BS, DI], bf16, kind="ExternalInput")
    ws_t = nc.dram_tensor("ws_bf", [M, DO, DI], bf16, kind="ExternalInput")
    pwT_t = nc.dram_tensor("pwT_bf", [M, BS], bf16, kind="ExternalInput")
    pbT_t = nc.dram_tensor("pbT_bf", [M, BS], bf16, kind="ExternalInput")
    bs_t = nc.dram_tensor("bs_bf", [M, DO], bf16, kind="ExternalInput")
    gw_t = nc.dram_tensor("gw_bf", [BS, DO], bf16, kind="ExternalInput")
    gb_t = nc.dram_tensor("gb_bf", [BS, DO], bf16, kind="ExternalInput")
    out_t = nc.dram_tensor("out", [BS, DO], f32, kind="ExternalOutput")

    with tile.TileContext(nc) as tc, ExitStack() as top:
        # pools that live for the whole kernel
        consts = top.enter_context(tc.tile_pool(name="consts", bufs=1))
        yT_pool = top.enter_context(tc.tile_pool(name="yT", bufs=KCH))
        ws_pool = top.enter_context(tc.tile_pool(name="ws", bufs=8))
        bbg_pool = top.enter_context(tc.tile_pool(name="bbg", bufs=NB))
        z_pool = top.enter_context(tc.tile_pool(name="z", bufs=NB))
        st_pool = top.enter_context(tc.tile_pool(name="stats", bufs=4 * NB))

        eps = consts.tile([P, 1], f32, tag="eps")
        nc.vector.memset(eps, LN_EPS)
        pbT_sb = consts.tile([P, BS], bf16, tag="pbT")
        nc.vector.memset(pbT_sb, 0.0)
        nc.sync.dma_start(out=pbT_sb[:M, :], in_=pbT_t[:, :])
        bs_sb = consts.tile([P, DO], bf16, tag="bs")
        nc.vector.memset(bs_sb, 0.0)
        nc.sync.dma_start(out=bs_sb[:M, :], in_=bs_t[:, :])

        bbg = [bbg_pool.tile([P, DO], bf16, tag="bbg", name="bbg") for _ in range(NB)]
        z_sb = [z_pool.tile([P, DO], bf16, tag="z", name="z") for _ in range(NB)]
        stats = [st_pool.tile([P, NC_I, 6], f32, tag="st", name="st") for _ in range(NB)]

        # ---- phase 0: bbg[b,i] = gb * (pb @ bs), and phase 1: build yT ----
        with ExitStack() as ph01:
            psum0 = ph01.enter_context(
                tc.tile_pool(name="psum0", bufs=2, space="PSUM")
            )
            gb0 = ph01.enter_context(tc.tile_pool(name="gb0", bufs=2))
            xT_pool = ph01.enter_context(tc.tile_pool(name="xT", bufs=JCH))
            pwm_pool = ph01.enter_context(tc.tile_pool(name="pwm", bufs=M))

            for bt in range(NB):
                gb_tile = gb0.tile([P, DO], bf16, tag="gb", name="gb")
                nc.scalar.dma_start(
                    out=gb_tile, in_=gb_t[bt * P : (bt + 1) * P, :]
                )
                for c in range(NC_I):
                    bbp = psum0.tile([P, 512], f32, tag="bbp", name="bbp")
                    nc.tensor.matmul(
                        bbp,
                        lhsT=pbT_sb[:, bt * P : (bt + 1) * P],
                        rhs=bs_sb[:, c * 512 : (c + 1) * 512],
                        start=True,
                        stop=True,
                    )
                    nc.vector.tensor_tensor(
                        out=bbg[bt][:, c * 512 : (c + 1) * 512],
                        in0=bbp,
                        in1=gb_tile[:, c * 512 : (c + 1) * 512],
                        op=OP.mult,
                    )

            xT = []
            for jc in range(JCH):
                xt = xT_pool.tile([P, BS], bf16, tag="xT", name="xT")
                nc.sync.dma_start_transpose(
                    out=xt, in_=x_t[:, jc * P : (jc + 1) * P]
                )
                xT.append(xt)
            pwm = []
            for m in range(M):
                pw = pwm_pool.tile([P, BS], bf16, tag="pwm", name="pwm")
                nc.gpsimd.dma_start(
                    out=pw, in_=pwT_t[m : m + 1, :].to_broadcast([P, BS])
                )
                pwm.append(pw)
            yT = []
            for m in range(M):
                for jc in range(JCH):
                    yt = yT_pool.tile([P, BS], bf16, tag="yT", name="yT")
                    nc.vector.tensor_tensor(
                        out=yt, in0=xT[jc], in1=pwm[m], op=OP.mult
                    )
                    yT.append(yt)

        # ---- phase 2: main accumulation, z = yT.T @ W2T, fused drain ----
        with ExitStack() as ph2:
            psum = ph2.enter_context(
                tc.tile_pool(name="psum", bufs=8, space="PSUM")
            )
            gw_pool = ph2.enter_context(tc.tile_pool(name="gw", bufs=8))
            tmp_pool = ph2.enter_context(tc.tile_pool(name="tmp", bufs=2))

            for c in range(NC_I):
                gw_sl = []
                for bt in range(NB):
                    g = gw_pool.tile([P, 512], bf16, tag="gw", name="gw")
                    nc.scalar.dma_start(
                        out=g,
                        in_=gw_t[
                            bt * P : (bt + 1) * P, c * 512 : (c + 1) * 512
                        ],
                    )
                    gw_sl.append(g)
                ps = [psum.tile([P, 512], f32, tag="mm", name="mm") for _ in range(NB)]
                for k in range(KCH):
                    m, jc = divmod(k, JCH)
                    wt = ws_pool.tile([P, 512], bf16, tag="ws", name="ws")
                    nc.sync.dma_start_transpose(
                        out=wt,
                        in_=ws_t[
                            m,
                            c * 512 : (c + 1) * 512,
                            jc * P : (jc + 1) * P,
                        ],
                    )
                    for bt in range(NB):
                        nc.tensor.matmul(
                            ps[bt],
                            lhsT=yT[k][:, bt * P : (bt + 1) * P],
                            rhs=wt,
                            start=(k == 0),
                            stop=(k == KCH - 1),
                        )
                for bt in range(NB):
                    zsl = z_sb[bt][:, c * 512 : (c + 1) * 512]
                    t = tmp_pool.tile([P, 512], f32, tag="tmp", name="tmp")
                    nc.vector.tensor_tensor(
                        out=t, in0=ps[bt], in1=gw_sl[bt], op=OP.mult
                    )
                    nc.vector.tensor_tensor(
                        out=zsl, in0=t, in1=bbg[bt][:, c * 512 : (c + 1) * 512],
                        op=OP.add,
                    )
                    nc.vector.bn_stats(out=stats[bt][:, c, :], in_=zsl)

        # ---- phase 3: LayerNorm apply + ELU + store ----
        with ExitStack() as ph3:
            small = ph3.enter_context(tc.tile_pool(name="small", bufs=4 * NB))
            y_pool = ph3.enter_context(tc.tile_pool(name="y", bufs=4))
            e_pool = ph3.enter_context(tc.tile_pool(name="e", bufs=4))
            o_pool = ph3.enter_context(tc.tile_pool(name="o", bufs=4))

            for bt in range(NB):
                mv = small.tile([P, 2], f32, tag="mv", name="mv")
                nc.vector.bn_aggr(out=mv, in_=stats[bt])
                std = small.tile([P, 1], f32, tag="std", name="std")
                nc.scalar.activation(
                    out=std, in_=mv[:, 1:2], func=FT.Sqrt, bias=eps
                )
                rstd = small.tile([P, 1], f32, tag="rstd", name="rstd")
                nc.vector.reciprocal(out=rstd, in_=std)
                nmr = small.tile([P, 1], f32, tag="nmr", name="nmr")
                nc.vector.scalar_tensor_tensor(
                    out=nmr, in0=mv[:, 0:1], scalar=-1.0, in1=rstd,
                    op0=OP.mult, op1=OP.mult,
                )
                for c in range(NC_I):
                    zsl = z_sb[bt][:, c * 512 : (c + 1) * 512]
                    ysl = y_pool.tile([P, 512], bf16, tag="y", name="y")
                    nc.vector.tensor_scalar(
                        out=ysl, in0=zsl, scalar1=mv[:, 0:1], scalar2=rstd,
                        op0=OP.subtract, op1=OP.mult,
                    )
                    esl = e_pool.tile([P, 512], bf16, tag="e", name="e")
                    nc.scalar.activation(
                        out=esl, in_=zsl, func=FT.Exp, bias=nmr, scale=rstd
                    )
                    nc.vector.tensor_scalar(
                        out=esl, in0=esl, scalar1=1.0, scalar2=-1.0,
                        op0=OP.min, op1=OP.add,
                    )
                    osl = o_pool.tile([P, 512], f32, tag="o", name="o")
                    nc.vector.scalar_tensor_tensor(
                        out=osl, in0=ysl, scalar=0.0, in1=esl,
                        op0=OP.max, op1=OP.add,
                    )
                    nc.scalar.dma_start(
                        out=out_t[bt * P : (bt + 1) * P, c * 512 : (c + 1) * 512],
                        in_=osl,
                    )
    nc.finalize()
    return nc


def _get_nc():
    if "nc" not in _cache:
        _cache["nc"] = _build_module()
    return _cache["nc"]


def _prep_inputs(x, Ws, bs, pW, pb, gW, gb):
    # cache the host-side bf16 conversions keyed on array identity (the
    # harness reuses the same input arrays across calls)
    key = tuple(id(a) for a in (x, Ws, bs, pW, pb, gW, gb))
    hit = _cache.get("prep")
    if hit is not None and hit[0] == key:
        return hit[2]
    x_bf = np.asarray(x, np.float32).astype(BF16)
    Ws_bf = np.asarray(Ws, np.float32).astype(BF16)
    gW_bf = np.asarray(gW, np.float32).astype(BF16)
    gb_bf = np.asarray(gb, np.float32).astype(BF16)
    pWT = np.ascontiguousarray(np.asarray(pW, np.float32).T).astype(BF16)
    pbT = np.ascontiguousarray(np.asarray(pb, np.float32).T).astype(BF16)
    bs_bf = np.asarray(bs, np.float32).astype(BF16)
    in_maps = []
    for c in range(NCORES):
        sl = slice(c * BS, (c + 1) * BS)
        in_maps.append(
            {
                "x_bf": x_bf[sl],
                "ws_bf": Ws_bf,
                "pwT_bf": np.ascontiguousarray(pWT[:, sl]),
                "pbT_bf": np.ascontiguousarray(pbT[:, sl]),
                "bs_bf": bs_bf,
                "gw_bf": gW_bf[sl],
                "gb_bf": gb_bf[sl],
            }
        )
    _cache["prep"] = (key, (x, Ws, bs, pW, pb, gW, gb), in_maps)
    return in_maps


def kernel(x, Ws, bs, pW, pb, gW, gb):
    from concourse import bass_utils

    nc = _get_nc()
    in_maps = _prep_inputs(x, Ws, bs, pW, pb, gW, gb)
    res = bass_utils.run_bass_kernel_spmd(
        nc, in_maps, core_ids=list(range(NCORES))
    )
    _cache["last_results"] = res
    return np.concatenate([r["out"] for r in res.results], axis=0)


def last_exec_time_ns():
    res = _cache.get("last_results")
    return None if res is None else res.exec_time_ns
